# revision 27
# baseline (speedup 1.0000x reference)
"""Trainium2 Bass kernel for the ContinuousVariableQNN problem.

Math reduction (validated against the jax reference on host):
  The reference builds a 256x256 symplectic matrix S from params, then
    mu   = mu0 @ S.T   with mu0[:, 0::2] = 2*inputs (odd cols zero)
    n    = (dsum + mu_x^2 + mu_p^2) / (2*hbar) - 0.5
  Because mu0's p-quadrature entries are all zero, the big matmul collapses to
    mu_dev = inputs @ Ms          with Ms[i, j] = S[j, 2*i]   ([128, 256])
  (factor 2 from displacement and the 1/4 normalization cancel), and
    n[b, m] = mu_dev[b, 2m]^2 + mu_dev[b, 2m+1]^2 + bias[m]
  with bias[m] = (diag(S S^T)[2m] + diag(S S^T)[2m+1])/4 - 0.5 (a constant).

Device strategy (pure data parallelism over 8 cores, batch-sharded):
  Per core: 16384 rows. For each 128-row tile:
    PE transpose X tile -> PSUM, DVE copy -> SBUF,
    PE matmul (fp32r)  XT.T @ Ms -> PSUM mu [128, 256],
    ACT Square -> SBUF, DVE pair-add (stride-2), GPSIMD add bias, DMA out.
  DMA layout puts CH consecutive batch rows on one partition so HBM
  transfers use multi-KB descriptors. Input DMAs ride the SP HWDGE queue,
  output DMAs the ACT HWDGE queue.
"""

import ml_dtypes
import numpy as np

import concourse.bass as bass
import concourse.mybir as mybir
import concourse.tile as tile
from concourse import bacc
from concourse.bass_utils import run_bass_kernel_spmd
from concourse.masks import make_identity

N_QUMODES = 128
N_LAYERS = 8
BATCH = 131072
N_CORES = 8
ROWS = BATCH // N_CORES          # 16384 rows per core
CH = 16                          # batch rows per partition per DMA chunk
CHUNK_ROWS = 128 * CH            # 2048
N_CHUNKS = ROWS // CHUNK_ROWS    # 8
SUB = 4                          # tiles (of 128 rows) per compute sub-chunk
F32 = mybir.dt.float32
F32R = mybir.dt.float32r
BF16 = mybir.dt.bfloat16


def host_prep(params: np.ndarray):
    """Build Ms [128, 256] and bias_rep [128, 512] on host (tiny, replicated)."""
    L, N = N_LAYERS, N_QUMODES
    p = params.reshape(L, N, 3).astype(np.float32)
    th1, r, th2 = p[..., 0], p[..., 1], p[..., 2]

    def rot(th):
        c, s = np.cos(th), np.sin(th)
        return np.stack([np.stack([c, -s], -1), np.stack([s, c], -1)], -2)

    z = np.zeros_like(r)
    sq = np.stack([np.stack([np.exp(-r), z], -1),
                   np.stack([z, np.exp(r)], -1)], -2)
    blk = np.einsum('lnab,lnbc,lncd->lnad', rot(th2), sq, rot(th1)).astype(np.float32)

    t = np.float32(np.cos(np.pi / 4))
    rr = np.float32(np.sin(np.pi / 4))
    BS4 = np.array([[t, 0., -rr, 0.],
                    [0., t, 0., -rr],
                    [rr, 0., t, 0.],
                    [0., rr, 0., t]], dtype=np.float32)
    C = np.eye(2 * N, dtype=np.float32)
    for i in range(N - 1):
        C[2 * i:2 * i + 4, :] = BS4 @ C[2 * i:2 * i + 4, :]

    S = np.eye(2 * N, dtype=np.float32)
    idx = np.arange(N)
    for l in range(L):
        D = np.zeros((N, 2, N, 2), np.float32)
        D[idx, :, idx, :] = blk[l]
        S = C @ (D.reshape(2 * N, 2 * N) @ S)

    # Natural interleaved column order: mu[b, 2m] = x_m, mu[b, 2m+1] = p_m.
    Ms = np.ascontiguousarray(S[:, 0::2].T, dtype=np.float32)      # [128, 256]

    dV = (S ** 2).sum(axis=1)                                      # [256]
    bias = ((dV[0::2] + dV[1::2]) / 4.0 - 0.5).astype(np.float32)  # [128]
    bias_rep = np.ascontiguousarray(
        np.tile(bias, (128, 2 * SUB)).astype(ml_dtypes.bfloat16))  # [128, 1024]
    ident = np.eye(128, dtype=np.float32)
    return Ms, bias_rep, ident


def build_bass():
    nc = bacc.Bacc("TRN2", target_bir_lowering=False, debug=False,
                   num_devices=N_CORES)

    x_d = nc.dram_tensor("x", [ROWS, 128], F32R, kind="ExternalInput")
    ms_d = nc.dram_tensor("ms", [128, 256], F32R, kind="ExternalInput")
    bias_d = nc.dram_tensor("bias_rep", [128, 2 * SUB * 128], BF16,
                            kind="ExternalInput")
    ident_d = nc.dram_tensor("ident", [128, 128], F32R, kind="ExternalInput")
    out_d = nc.dram_tensor("out", [ROWS, 128], F32, kind="ExternalOutput")

    x_v = x_d.ap().rearrange("(c p r) i -> c p r i", p=128, r=CH)
    out_v = out_d.ap().rearrange("(c p r) m -> c p r m", p=128, r=CH)

    with tile.TileContext(nc) as tc:
        with (
            tc.tile_pool(name="const", bufs=1) as const_pool,
            tc.tile_pool(name="xin", bufs=3) as xin_pool,
            tc.tile_pool(name="oout", bufs=3) as oout_pool,
            tc.tile_pool(name="xts", bufs=3) as xts_pool,
            tc.tile_pool(name="sq", bufs=3) as sq_pool,
            tc.tile_pool(name="tmp", bufs=3) as tmp_pool,
            tc.tile_pool(name="xtp", bufs=2, space="PSUM") as xtp_pool,
            tc.tile_pool(name="mup", bufs=3, space="PSUM") as mup_pool,
        ):
            # Get the first input chunk moving before anything else.
            x_first = xin_pool.tile([128, CH, 128], F32R, tag="x_sb")
            nc.sync.dma_start(out=x_first, in_=x_v[0])

            ident = const_pool.tile([128, 128], F32R)
            nc.sync.dma_start(out=ident, in_=ident_d.ap())
            ms_sb = const_pool.tile([128, 256], F32R)
            nc.sync.dma_start(out=ms_sb, in_=ms_d.ap())
            bias_sb = const_pool.tile([128, 2 * SUB * 128], BF16)
            nc.sync.dma_start(out=bias_sb, in_=bias_d.ap())

            for c in range(N_CHUNKS):
                if c == 0:
                    x_sb = x_first
                else:
                    x_sb = xin_pool.tile([128, CH, 128], F32R, tag="x_sb")
                    nc.sync.dma_start(out=x_sb, in_=x_v[c])
                out_sb = oout_pool.tile([128, CH, 128], F32)

                for h in range(CH // (2 * SUB)):
                    tmp_sb = tmp_pool.tile([128, 2, SUB, 128], BF16)
                    for s2 in range(2):
                        s = 2 * h + s2
                        xt_ps = xtp_pool.tile([128, SUB, 128], F32R)   # 1 bank
                        mu_ps = mup_pool.tile([128, SUB, 256], F32)    # 2 banks
                        xt_sb = xts_pool.tile([128, SUB, 128], F32R)
                        sq_sb = sq_pool.tile([128, SUB, 256], BF16)

                        for q in range(SUB):
                            nc.tensor.transpose(xt_ps[:, q, :],
                                                x_sb[:, SUB * s + q, :], ident)
                        nc.vector.tensor_copy(xt_sb, xt_ps)
                        for q in range(SUB):
                            nc.tensor.matmul(mu_ps[:, q, :],
                                             xt_sb[:, q, :], ms_sb,
                                             start=True, stop=True)
                        # Square with a de-interleaving AP pair: iterate
                        # (q, h, m); reads walk mu x/p interleaved (stride 2),
                        # writes land [x-half | p-half] so the pair-add reads
                        # contiguous halves.
                        mu_v = mu_ps.rearrange("p a b -> p (a b)").rearrange(
                            "p (q m e) -> p q e m", q=SUB, e=2)
                        sq_flat = sq_sb.rearrange("p a b -> p (a b)")
                        sq_v = sq_flat.rearrange(
                            "p (e q m) -> p q e m", e=2, q=SUB)
                        nc.scalar.activation(sq_v, mu_v,
                                             mybir.ActivationFunctionType.Square)
                        tmp_flat = tmp_sb[:, s2].rearrange("p a b -> p (a b)")
                        nc.vector.tensor_tensor(out=tmp_flat,
                                                in0=sq_flat[:, 0:SUB * 128],
                                                in1=sq_flat[:, SUB * 128:],
                                                op=mybir.AluOpType.add)
                    nc.gpsimd.tensor_tensor(
                        out=out_sb[:, 2 * SUB * h:2 * SUB * (h + 1), :]
                            .rearrange("p a b -> p (a b)"),
                        in0=tmp_sb.rearrange("p a s b -> p (a s b)"),
                        in1=bias_sb,
                        op=mybir.AluOpType.add)

                nc.sync.dma_start(out=out_v[c], in_=out_sb)

    nc.compile()
    return nc


_NC_CACHE = None


def kernel(**inputs: np.ndarray) -> np.ndarray:
    global _NC_CACHE
    X = np.ascontiguousarray(np.asarray(inputs["inputs"], dtype=np.float32))
    params = np.asarray(inputs["params"], dtype=np.float32)
    assert X.shape == (BATCH, N_QUMODES)

    Ms, bias_rep, ident = host_prep(params)

    if _NC_CACHE is None:
        _NC_CACHE = build_bass()
    nc = _NC_CACHE

    in_maps = [
        {"x": X[i * ROWS:(i + 1) * ROWS], "ms": Ms, "bias_rep": bias_rep,
         "ident": ident}
        for i in range(N_CORES)
    ]
    res = run_bass_kernel_spmd(nc, in_maps, core_ids=list(range(N_CORES)))
    out = np.concatenate([r["out"] for r in res.results], axis=0)
    return out.astype(np.float32)


# revision 31
# speedup vs baseline: 1.0703x; 1.0703x over previous
"""Trainium2 Bass kernel for the ContinuousVariableQNN problem.

Math reduction (validated against the jax reference on host):
  The reference builds a 256x256 symplectic matrix S from params, then
    mu   = mu0 @ S.T   with mu0[:, 0::2] = 2*inputs (odd cols zero)
    n    = (dsum + mu_x^2 + mu_p^2) / (2*hbar) - 0.5
  Because mu0's p-quadrature entries are all zero, the big matmul collapses to
    mu_dev = inputs @ Ms          with Ms[i, j] = S[j, 2*i]   ([128, 256])
  (factor 2 from displacement and the 1/4 normalization cancel), and
    n[b, m] = mu_dev[b, 2m]^2 + mu_dev[b, 2m+1]^2 + bias[m]
  with bias[m] = (diag(S S^T)[2m] + diag(S S^T)[2m+1])/4 - 0.5 (a constant).

Device strategy (pure data parallelism over 8 cores, batch-sharded):
  Per core: 16384 rows. For each 128-row tile:
    PE transpose X tile -> PSUM, DVE copy -> SBUF,
    PE matmul (fp32r)  XT.T @ Ms -> PSUM mu [128, 256],
    ACT Square -> SBUF, DVE pair-add (stride-2), GPSIMD add bias, DMA out.
  DMA layout puts CH consecutive batch rows on one partition so HBM
  transfers use multi-KB descriptors. Input DMAs ride the SP HWDGE queue,
  output DMAs the ACT HWDGE queue.
"""

import ml_dtypes
import numpy as np

import concourse.bass as bass
import concourse.mybir as mybir
import concourse.tile as tile
from concourse import bacc
from concourse.bass_utils import run_bass_kernel_spmd
from concourse.masks import make_identity

N_QUMODES = 128
N_LAYERS = 8
BATCH = 131072
N_CORES = 8
ROWS = BATCH // N_CORES          # 16384 rows per core
CH = 8                           # batch rows per partition per DMA chunk
CHUNK_ROWS = 128 * CH            # 1024
N_CHUNKS = ROWS // CHUNK_ROWS    # 16
SUBS_PER_CHUNK = CH // 4         # 2
N_SUBS = N_CHUNKS * SUBS_PER_CHUNK
SUB = 4                          # tiles (of 128 rows) per compute sub-chunk
F32 = mybir.dt.float32
F32R = mybir.dt.float32r
BF16 = mybir.dt.bfloat16


def host_prep(params: np.ndarray):
    """Build Ms [128, 256] and bias_rep [128, 512] on host (tiny, replicated)."""
    L, N = N_LAYERS, N_QUMODES
    p = params.reshape(L, N, 3).astype(np.float32)
    th1, r, th2 = p[..., 0], p[..., 1], p[..., 2]

    def rot(th):
        c, s = np.cos(th), np.sin(th)
        return np.stack([np.stack([c, -s], -1), np.stack([s, c], -1)], -2)

    z = np.zeros_like(r)
    sq = np.stack([np.stack([np.exp(-r), z], -1),
                   np.stack([z, np.exp(r)], -1)], -2)
    blk = np.einsum('lnab,lnbc,lncd->lnad', rot(th2), sq, rot(th1)).astype(np.float32)

    t = np.float32(np.cos(np.pi / 4))
    rr = np.float32(np.sin(np.pi / 4))
    BS4 = np.array([[t, 0., -rr, 0.],
                    [0., t, 0., -rr],
                    [rr, 0., t, 0.],
                    [0., rr, 0., t]], dtype=np.float32)
    C = np.eye(2 * N, dtype=np.float32)
    for i in range(N - 1):
        C[2 * i:2 * i + 4, :] = BS4 @ C[2 * i:2 * i + 4, :]

    S = np.eye(2 * N, dtype=np.float32)
    idx = np.arange(N)
    for l in range(L):
        D = np.zeros((N, 2, N, 2), np.float32)
        D[idx, :, idx, :] = blk[l]
        S = C @ (D.reshape(2 * N, 2 * N) @ S)

    # Natural interleaved column order: mu[b, 2m] = x_m, mu[b, 2m+1] = p_m.
    Ms = np.ascontiguousarray(S[:, 0::2].T, dtype=np.float32)      # [128, 256]

    dV = (S ** 2).sum(axis=1)                                      # [256]
    bias = ((dV[0::2] + dV[1::2]) / 4.0 - 0.5).astype(np.float32)  # [128]
    bias_rep = np.ascontiguousarray(
        np.tile(bias, (128, SUB)).astype(ml_dtypes.bfloat16))      # [128, 512]
    ident = np.eye(128, dtype=np.float32)
    return Ms, bias_rep, ident


def build_bass():
    nc = bacc.Bacc("TRN2", target_bir_lowering=False, debug=False,
                   num_devices=N_CORES)

    x_d = nc.dram_tensor("x", [ROWS, 128], F32R, kind="ExternalInput")
    ms_d = nc.dram_tensor("ms", [128, 256], F32R, kind="ExternalInput")
    bias_d = nc.dram_tensor("bias_rep", [128, SUB * 128], BF16,
                            kind="ExternalInput")
    ident_d = nc.dram_tensor("ident", [128, 128], F32R, kind="ExternalInput")
    out_d = nc.dram_tensor("out", [ROWS, 128], F32, kind="ExternalOutput")

    x_v = x_d.ap().rearrange("(c p r) i -> c p r i", p=128, r=CH)
    out_v = out_d.ap().rearrange("(c p r) m -> c p r m", p=128, r=CH)

    with tile.TileContext(nc) as tc:
        with (
            tc.tile_pool(name="const", bufs=1) as const_pool,
            tc.tile_pool(name="xin", bufs=3) as xin_pool,
            tc.tile_pool(name="oout", bufs=3) as oout_pool,
            tc.tile_pool(name="xts", bufs=3) as xts_pool,
            tc.tile_pool(name="sq", bufs=3) as sq_pool,
            tc.tile_pool(name="tmp", bufs=3) as tmp_pool,
            tc.tile_pool(name="xtp", bufs=2, space="PSUM") as xtp_pool,
            tc.tile_pool(name="mup", bufs=3, space="PSUM") as mup_pool,
        ):
            ident = const_pool.tile([128, 128], F32R)
            nc.sync.dma_start(out=ident, in_=ident_d.ap())

            # First input chunk next on the queue, then the remaining consts.
            x_tiles: dict[int, bass.AP] = {}
            out_tiles: dict[int, bass.AP] = {}
            xt_tiles: dict[int, bass.AP] = {}
            mu_tiles: dict[int, bass.AP] = {}
            sq_tiles: dict[int, bass.AP] = {}

            def load_chunk(c):
                x_sb = xin_pool.tile([128, CH, 128], F32R, tag="x_sb",
                                     name=f"x_sb_{c}")
                nc.sync.dma_start(out=x_sb, in_=x_v[c])
                x_tiles[c] = x_sb
                out_tiles[c] = oout_pool.tile([128, CH, 128], F32, tag="o_sb",
                                              name=f"o_sb_{c}")

            load_chunk(0)
            ms_sb = const_pool.tile([128, 256], F32R)
            nc.sync.dma_start(out=ms_sb, in_=ms_d.ap())
            bias_sb = const_pool.tile([128, SUB * 128], BF16)
            nc.sync.dma_start(out=bias_sb, in_=bias_d.ap())

            # Software-pipelined over sub-chunks: transposes run one stage
            # ahead of the matmuls and two ahead of the elementwise tail so
            # the PE's in-order queue never waits on the DVE copy.
            for i in range(N_SUBS + 2):
                # stage A: transposes + PSUM->SBUF copy for sub-chunk i
                if i < N_SUBS:
                    c, sc = divmod(i, SUBS_PER_CHUNK)
                    if sc == 0 and c + 1 < N_CHUNKS:
                        load_chunk(c + 1)
                    x_sb = x_tiles[c]
                    xt_ps = xtp_pool.tile([128, SUB, 128], F32R)     # 1 bank
                    for q in range(SUB):
                        nc.tensor.transpose(xt_ps[:, q, :],
                                            x_sb[:, SUB * sc + q, :], ident)
                    xt_sb = xts_pool.tile([128, SUB, 128], F32R)
                    nc.vector.tensor_copy(xt_sb, xt_ps)
                    xt_tiles[i] = xt_sb

                # stage B: matmuls + square for sub-chunk i-1
                t = i - 1
                if 0 <= t < N_SUBS:
                    xt_sb = xt_tiles.pop(t)
                    mu_ps = mup_pool.tile([128, SUB, 256], F32)      # 2 banks
                    for q in range(SUB):
                        nc.tensor.matmul(mu_ps[:, q, :],
                                         xt_sb[:, q, :], ms_sb,
                                         start=True, stop=True)
                    sq_sb = sq_pool.tile([128, SUB, 256], BF16)
                    # De-interleaving AP pair: reads walk mu x/p interleaved
                    # (stride 2), writes land [x-half | p-half] so the
                    # pair-add reads contiguous halves.
                    mu_v = mu_ps.rearrange("p a b -> p (a b)").rearrange(
                        "p (q m e) -> p q e m", q=SUB, e=2)
                    sq_v = sq_sb.rearrange("p a b -> p (a b)").rearrange(
                        "p (e q m) -> p q e m", e=2, q=SUB)
                    nc.scalar.activation(sq_v, mu_v,
                                         mybir.ActivationFunctionType.Square)
                    mu_tiles[t] = mu_ps
                    sq_tiles[t] = sq_sb

                # stage C: pair-add + bias + output DMA for sub-chunk i-2
                u = i - 2
                if u >= 0:
                    cu, scu = divmod(u, SUBS_PER_CHUNK)
                    mu_tiles.pop(u, None)
                    sq_sb = sq_tiles.pop(u)
                    sq_flat = sq_sb.rearrange("p a b -> p (a b)")
                    tmp_sb = tmp_pool.tile([128, SUB, 128], BF16)
                    tmp_flat = tmp_sb.rearrange("p a b -> p (a b)")
                    nc.vector.tensor_tensor(out=tmp_flat,
                                            in0=sq_flat[:, 0:SUB * 128],
                                            in1=sq_flat[:, SUB * 128:],
                                            op=mybir.AluOpType.add)
                    nc.gpsimd.tensor_tensor(
                        out=out_tiles[cu][:, SUB * scu:SUB * (scu + 1), :],
                        in0=tmp_sb, in1=bias_sb,
                        op=mybir.AluOpType.add)
                    if scu == SUBS_PER_CHUNK - 1:
                        nc.sync.dma_start(out=out_v[cu], in_=out_tiles.pop(cu))
                        x_tiles.pop(cu, None)

    nc.compile()
    return nc


_NC_CACHE = None


def kernel(**inputs: np.ndarray) -> np.ndarray:
    global _NC_CACHE
    X = np.ascontiguousarray(np.asarray(inputs["inputs"], dtype=np.float32))
    params = np.asarray(inputs["params"], dtype=np.float32)
    assert X.shape == (BATCH, N_QUMODES)

    Ms, bias_rep, ident = host_prep(params)

    if _NC_CACHE is None:
        _NC_CACHE = build_bass()
    nc = _NC_CACHE

    in_maps = [
        {"x": X[i * ROWS:(i + 1) * ROWS], "ms": Ms, "bias_rep": bias_rep,
         "ident": ident}
        for i in range(N_CORES)
    ]
    res = run_bass_kernel_spmd(nc, in_maps, core_ids=list(range(N_CORES)))
    out = np.concatenate([r["out"] for r in res.results], axis=0)
    return out.astype(np.float32)


# revision 33
# speedup vs baseline: 1.1810x; 1.1034x over previous
"""Trainium2 Bass kernel for the ContinuousVariableQNN problem.

Math reduction (validated against the jax reference on host):
  The reference builds a 256x256 symplectic matrix S from params, then
    mu   = mu0 @ S.T   with mu0[:, 0::2] = 2*inputs (odd cols zero)
    n    = (dsum + mu_x^2 + mu_p^2) / (2*hbar) - 0.5
  Because mu0's p-quadrature entries are all zero, the big matmul collapses to
    mu_dev = inputs @ Ms          with Ms[i, j] = S[j, 2*i]   ([128, 256])
  (factor 2 from displacement and the 1/4 normalization cancel), and
    n[b, m] = mu_dev[b, 2m]^2 + mu_dev[b, 2m+1]^2 + bias[m]
  with bias[m] = (diag(S S^T)[2m] + diag(S S^T)[2m+1])/4 - 0.5 (a constant).

Device strategy (pure data parallelism over 8 cores, batch-sharded):
  Per core: 16384 rows. For each 128-row tile:
    PE transpose X tile -> PSUM, DVE copy -> SBUF,
    PE matmul (fp32r)  XT.T @ Ms -> PSUM mu [128, 256],
    ACT Square -> SBUF, DVE pair-add (stride-2), GPSIMD add bias, DMA out.
  DMA layout puts CH consecutive batch rows on one partition so HBM
  transfers use multi-KB descriptors. Input DMAs ride the SP HWDGE queue,
  output DMAs the ACT HWDGE queue.
"""

import ml_dtypes
import numpy as np

import concourse.bass as bass
import concourse.mybir as mybir
import concourse.tile as tile
from concourse import bacc
from concourse.bass_utils import run_bass_kernel_spmd
from concourse.masks import make_identity

N_QUMODES = 128
N_LAYERS = 8
BATCH = 131072
N_CORES = 8
ROWS = BATCH // N_CORES          # 16384 rows per core
CH = 16                          # batch rows per partition per DMA chunk
CHUNK_ROWS = 128 * CH            # 2048
N_CHUNKS = ROWS // CHUNK_ROWS    # 8
SUBS_PER_CHUNK = CH // 4         # 4
N_SUBS = N_CHUNKS * SUBS_PER_CHUNK
SUB = 4                          # tiles (of 128 rows) per compute sub-chunk
F32 = mybir.dt.float32
F32R = mybir.dt.float32r
BF16 = mybir.dt.bfloat16


def host_prep(params: np.ndarray):
    """Build Ms [128, 256] and bias_rep [128, 512] on host (tiny, replicated)."""
    L, N = N_LAYERS, N_QUMODES
    p = params.reshape(L, N, 3).astype(np.float32)
    th1, r, th2 = p[..., 0], p[..., 1], p[..., 2]

    def rot(th):
        c, s = np.cos(th), np.sin(th)
        return np.stack([np.stack([c, -s], -1), np.stack([s, c], -1)], -2)

    z = np.zeros_like(r)
    sq = np.stack([np.stack([np.exp(-r), z], -1),
                   np.stack([z, np.exp(r)], -1)], -2)
    blk = np.einsum('lnab,lnbc,lncd->lnad', rot(th2), sq, rot(th1)).astype(np.float32)

    t = np.float32(np.cos(np.pi / 4))
    rr = np.float32(np.sin(np.pi / 4))
    BS4 = np.array([[t, 0., -rr, 0.],
                    [0., t, 0., -rr],
                    [rr, 0., t, 0.],
                    [0., rr, 0., t]], dtype=np.float32)
    C = np.eye(2 * N, dtype=np.float32)
    for i in range(N - 1):
        C[2 * i:2 * i + 4, :] = BS4 @ C[2 * i:2 * i + 4, :]

    S = np.eye(2 * N, dtype=np.float32)
    idx = np.arange(N)
    for l in range(L):
        D = np.zeros((N, 2, N, 2), np.float32)
        D[idx, :, idx, :] = blk[l]
        S = C @ (D.reshape(2 * N, 2 * N) @ S)

    # Natural interleaved column order: mu[b, 2m] = x_m, mu[b, 2m+1] = p_m.
    Ms = np.ascontiguousarray(S[:, 0::2].T, dtype=np.float32)      # [128, 256]

    dV = (S ** 2).sum(axis=1)                                      # [256]
    bias = ((dV[0::2] + dV[1::2]) / 4.0 - 0.5).astype(np.float32)  # [128]
    bias_rep = np.ascontiguousarray(
        np.tile(bias, (128, SUB)).astype(ml_dtypes.bfloat16))      # [128, 512]
    ident = np.eye(128, dtype=np.float32)
    return Ms, bias_rep, ident


def build_bass():
    nc = bacc.Bacc("TRN2", target_bir_lowering=False, debug=False,
                   num_devices=N_CORES)

    x_d = nc.dram_tensor("x", [ROWS, 128], F32R, kind="ExternalInput")
    ms_d = nc.dram_tensor("ms", [128, 256], F32R, kind="ExternalInput")
    bias_d = nc.dram_tensor("bias_rep", [128, SUB * 128], BF16,
                            kind="ExternalInput")
    ident_d = nc.dram_tensor("ident", [128, 128], F32R, kind="ExternalInput")
    out_d = nc.dram_tensor("out", [ROWS, 128], F32, kind="ExternalOutput")

    x_v = x_d.ap().rearrange("(c p r) i -> c p r i", p=128, r=CH)
    out_v = out_d.ap().rearrange("(c p r) m -> c p r m", p=128, r=CH)

    with tile.TileContext(nc) as tc:
        with (
            tc.tile_pool(name="const", bufs=1) as const_pool,
            tc.tile_pool(name="xin", bufs=3) as xin_pool,
            tc.tile_pool(name="oout", bufs=3) as oout_pool,
            tc.tile_pool(name="xts", bufs=3) as xts_pool,
            tc.tile_pool(name="sq", bufs=3) as sq_pool,
            tc.tile_pool(name="tmp", bufs=3) as tmp_pool,
            tc.tile_pool(name="xtp", bufs=2, space="PSUM") as xtp_pool,
            tc.tile_pool(name="mup", bufs=3, space="PSUM") as mup_pool,
        ):
            ident = const_pool.tile([128, 128], F32R)
            nc.sync.dma_start(out=ident, in_=ident_d.ap())

            # First input chunk next on the queue, then the remaining consts.
            x_tiles: dict[int, bass.AP] = {}
            out_tiles: dict[int, bass.AP] = {}
            xt_tiles: dict[int, bass.AP] = {}
            mu_tiles: dict[int, bass.AP] = {}
            sq_tiles: dict[int, bass.AP] = {}

            def load_chunk(c):
                x_sb = xin_pool.tile([128, CH, 128], F32R, tag="x_sb",
                                     name=f"x_sb_{c}")
                nc.sync.dma_start(out=x_sb, in_=x_v[c])
                x_tiles[c] = x_sb
                out_tiles[c] = oout_pool.tile([128, CH, 128], F32, tag="o_sb",
                                              name=f"o_sb_{c}")

            load_chunk(0)
            ms_sb = const_pool.tile([128, 256], F32R)
            nc.sync.dma_start(out=ms_sb, in_=ms_d.ap())
            bias_sb = const_pool.tile([128, SUB * 128], BF16)
            nc.sync.dma_start(out=bias_sb, in_=bias_d.ap())

            # Software-pipelined over sub-chunks: transposes run one stage
            # ahead of the matmuls and two ahead of the elementwise tail so
            # the PE's in-order queue never waits on the DVE copy.
            for i in range(N_SUBS + 2):
                # stage A: transposes + PSUM->SBUF copy for sub-chunk i
                if i < N_SUBS:
                    c, sc = divmod(i, SUBS_PER_CHUNK)
                    if sc == 0 and c + 1 < N_CHUNKS:
                        load_chunk(c + 1)
                    x_sb = x_tiles[c]
                    xt_ps = xtp_pool.tile([128, SUB, 128], F32R)     # 1 bank
                    for q in range(SUB):
                        nc.tensor.transpose(xt_ps[:, q, :],
                                            x_sb[:, SUB * sc + q, :], ident)
                    xt_sb = xts_pool.tile([128, SUB, 128], F32R)
                    # Alternate the PSUM->SBUF copy between DVE and ACT to
                    # keep both below the DMA pace.
                    if i % 2 == 0:
                        nc.vector.tensor_copy(xt_sb, xt_ps)
                    else:
                        nc.scalar.copy(xt_sb, xt_ps)
                    xt_tiles[i] = xt_sb

                # stage B: matmuls + square for sub-chunk i-1
                t = i - 1
                if 0 <= t < N_SUBS:
                    xt_sb = xt_tiles.pop(t)
                    mu_ps = mup_pool.tile([128, SUB, 256], F32)      # 2 banks
                    for q in range(SUB):
                        nc.tensor.matmul(mu_ps[:, q, :],
                                         xt_sb[:, q, :], ms_sb,
                                         start=True, stop=True)
                    sq_sb = sq_pool.tile([128, SUB, 256], BF16)
                    # De-interleaving AP pair: reads walk mu x/p interleaved
                    # (stride 2), writes land [x-half | p-half] so the
                    # pair-add reads contiguous halves.
                    mu_v = mu_ps.rearrange("p a b -> p (a b)").rearrange(
                        "p (q m e) -> p q e m", q=SUB, e=2)
                    sq_v = sq_sb.rearrange("p a b -> p (a b)").rearrange(
                        "p (e q m) -> p q e m", e=2, q=SUB)
                    nc.scalar.activation(sq_v, mu_v,
                                         mybir.ActivationFunctionType.Square)
                    mu_tiles[t] = mu_ps
                    sq_tiles[t] = sq_sb

                # stage C: pair-add + bias + output DMA for sub-chunk i-2
                u = i - 2
                if u >= 0:
                    cu, scu = divmod(u, SUBS_PER_CHUNK)
                    mu_tiles.pop(u, None)
                    sq_sb = sq_tiles.pop(u)
                    sq_flat = sq_sb.rearrange("p a b -> p (a b)")
                    tmp_sb = tmp_pool.tile([128, SUB, 128], BF16)
                    tmp_flat = tmp_sb.rearrange("p a b -> p (a b)")
                    nc.vector.tensor_tensor(out=tmp_flat,
                                            in0=sq_flat[:, 0:SUB * 128],
                                            in1=sq_flat[:, SUB * 128:],
                                            op=mybir.AluOpType.add)
                    nc.gpsimd.tensor_tensor(
                        out=out_tiles[cu][:, SUB * scu:SUB * (scu + 1), :],
                        in0=tmp_sb, in1=bias_sb,
                        op=mybir.AluOpType.add)
                    if scu == SUBS_PER_CHUNK - 1:
                        nc.sync.dma_start(out=out_v[cu], in_=out_tiles.pop(cu))
                        x_tiles.pop(cu, None)

    nc.compile()
    return nc


_NC_CACHE = None


def kernel(**inputs: np.ndarray) -> np.ndarray:
    global _NC_CACHE
    X = np.ascontiguousarray(np.asarray(inputs["inputs"], dtype=np.float32))
    params = np.asarray(inputs["params"], dtype=np.float32)
    assert X.shape == (BATCH, N_QUMODES)

    Ms, bias_rep, ident = host_prep(params)

    if _NC_CACHE is None:
        _NC_CACHE = build_bass()
    nc = _NC_CACHE

    in_maps = [
        {"x": X[i * ROWS:(i + 1) * ROWS], "ms": Ms, "bias_rep": bias_rep,
         "ident": ident}
        for i in range(N_CORES)
    ]
    res = run_bass_kernel_spmd(nc, in_maps, core_ids=list(range(N_CORES)))
    out = np.concatenate([r["out"] for r in res.results], axis=0)
    return out.astype(np.float32)


# revision 39
# speedup vs baseline: 1.2342x; 1.0451x over previous
"""Trainium2 Bass kernel for the ContinuousVariableQNN problem.

Math reduction (validated against the jax reference on host):
  The reference builds a 256x256 symplectic matrix S from params, then
    mu   = mu0 @ S.T   with mu0[:, 0::2] = 2*inputs (odd cols zero)
    n    = (dsum + mu_x^2 + mu_p^2) / (2*hbar) - 0.5
  Because mu0's p-quadrature entries are all zero, the big matmul collapses to
    mu_dev = inputs @ Ms          with Ms[i, j] = S[j, 2*i]   ([128, 256])
  (factor 2 from displacement and the 1/4 normalization cancel), and
    n[b, m] = mu_dev[b, 2m]^2 + mu_dev[b, 2m+1]^2 + bias[m]
  with bias[m] = (diag(S S^T)[2m] + diag(S S^T)[2m+1])/4 - 0.5 (a constant).

Device strategy (pure data parallelism over 8 cores, batch-sharded):
  Per core: 16384 rows. For each 128-row tile:
    PE transpose X tile -> PSUM, DVE copy -> SBUF,
    PE matmul (fp32r)  XT.T @ Ms -> PSUM mu [128, 256],
    ACT Square -> SBUF, DVE pair-add (stride-2), GPSIMD add bias, DMA out.
  DMA layout puts CH consecutive batch rows on one partition so HBM
  transfers use multi-KB descriptors. Input DMAs ride the SP HWDGE queue,
  output DMAs the ACT HWDGE queue.
"""

import ml_dtypes
import numpy as np

import concourse.bass as bass
import concourse.mybir as mybir
import concourse.tile as tile
from concourse import bacc
from concourse.bass_utils import run_bass_kernel_spmd
from concourse.masks import make_identity

N_QUMODES = 128
N_LAYERS = 8
BATCH = 131072
N_CORES = 8
ROWS = BATCH // N_CORES          # 16384 rows per core
CH = 16                          # batch rows per partition per DMA chunk
CHUNK_ROWS = 128 * CH            # 2048
N_CHUNKS = ROWS // CHUNK_ROWS    # 8
SUBS_PER_CHUNK = CH // 4         # 4
N_SUBS = N_CHUNKS * SUBS_PER_CHUNK
SUB = 4                          # tiles (of 128 rows) per compute sub-chunk
F32 = mybir.dt.float32
F32R = mybir.dt.float32r
BF16 = mybir.dt.bfloat16


def host_prep(params: np.ndarray):
    """Build Ms [128, 256] and bias_rep [128, 512] on host (tiny, replicated)."""
    L, N = N_LAYERS, N_QUMODES
    p = params.reshape(L, N, 3).astype(np.float32)
    th1, r, th2 = p[..., 0], p[..., 1], p[..., 2]

    def rot(th):
        c, s = np.cos(th), np.sin(th)
        return np.stack([np.stack([c, -s], -1), np.stack([s, c], -1)], -2)

    z = np.zeros_like(r)
    sq = np.stack([np.stack([np.exp(-r), z], -1),
                   np.stack([z, np.exp(r)], -1)], -2)
    blk = np.einsum('lnab,lnbc,lncd->lnad', rot(th2), sq, rot(th1)).astype(np.float32)

    t = np.float32(np.cos(np.pi / 4))
    rr = np.float32(np.sin(np.pi / 4))
    BS4 = np.array([[t, 0., -rr, 0.],
                    [0., t, 0., -rr],
                    [rr, 0., t, 0.],
                    [0., rr, 0., t]], dtype=np.float32)
    C = np.eye(2 * N, dtype=np.float32)
    for i in range(N - 1):
        C[2 * i:2 * i + 4, :] = BS4 @ C[2 * i:2 * i + 4, :]

    S = np.eye(2 * N, dtype=np.float32)
    idx = np.arange(N)
    for l in range(L):
        D = np.zeros((N, 2, N, 2), np.float32)
        D[idx, :, idx, :] = blk[l]
        S = C @ (D.reshape(2 * N, 2 * N) @ S)

    # Natural interleaved column order: mu[b, 2m] = x_m, mu[b, 2m+1] = p_m.
    Ms = np.ascontiguousarray(S[:, 0::2].T, dtype=np.float32)      # [128, 256]

    dV = (S ** 2).sum(axis=1)                                      # [256]
    bias = ((dV[0::2] + dV[1::2]) / 4.0 - 0.5).astype(np.float32)  # [128]
    bias_rep = np.ascontiguousarray(
        np.tile(bias, (128, SUB)).astype(ml_dtypes.bfloat16))      # [128, 512]
    ident = np.eye(128, dtype=np.float32)
    return Ms, bias_rep, ident


def build_bass():
    nc = bacc.Bacc("TRN2", target_bir_lowering=False, debug=False,
                   num_devices=N_CORES)

    x_d = nc.dram_tensor("x", [ROWS, 128], F32R, kind="ExternalInput")
    ms_d = nc.dram_tensor("ms", [128, 256], F32R, kind="ExternalInput")
    bias_d = nc.dram_tensor("bias_rep", [128, SUB * 128], BF16,
                            kind="ExternalInput")
    ident_d = nc.dram_tensor("ident", [128, 128], F32R, kind="ExternalInput")
    out_d = nc.dram_tensor("out", [ROWS, 128], F32, kind="ExternalOutput")

    x_v = x_d.ap().rearrange("(c p r) i -> c p r i", p=128, r=CH)
    out_v = out_d.ap().rearrange("(c p r) m -> c p r m", p=128, r=CH)

    with tile.TileContext(nc) as tc:
        with (
            tc.tile_pool(name="const", bufs=1) as const_pool,
            tc.tile_pool(name="xin", bufs=3) as xin_pool,
            tc.tile_pool(name="oout", bufs=3) as oout_pool,
            tc.tile_pool(name="xts", bufs=4) as xts_pool,
            tc.tile_pool(name="sq", bufs=4) as sq_pool,
            tc.tile_pool(name="tmp", bufs=4) as tmp_pool,
            tc.tile_pool(name="xtp", bufs=2, space="PSUM") as xtp_pool,
            tc.tile_pool(name="mup", bufs=3, space="PSUM") as mup_pool,
        ):
            ident = const_pool.tile([128, 128], F32R)
            nc.sync.dma_start(out=ident, in_=ident_d.ap())

            # First input chunk next on the queue, then the remaining consts.
            x_tiles: dict[int, bass.AP] = {}
            out_tiles: dict[int, bass.AP] = {}
            xt_tiles: dict[int, bass.AP] = {}
            mu_tiles: dict[int, bass.AP] = {}
            sq_tiles: dict[int, bass.AP] = {}

            def load_chunk(c):
                x_sb = xin_pool.tile([128, CH, 128], F32R, tag="x_sb",
                                     name=f"x_sb_{c}")
                if c == 0:
                    # halve the first transfer so the PE can start sooner
                    nc.sync.dma_start(out=x_sb[:, 0:CH // 2, :],
                                      in_=x_v[c][:, 0:CH // 2, :])
                    nc.sync.dma_start(out=x_sb[:, CH // 2:, :],
                                      in_=x_v[c][:, CH // 2:, :])
                else:
                    nc.sync.dma_start(out=x_sb, in_=x_v[c])
                x_tiles[c] = x_sb
                out_tiles[c] = oout_pool.tile([128, CH, 128], F32, tag="o_sb",
                                              name=f"o_sb_{c}")

            load_chunk(0)
            ms_sb = const_pool.tile([128, 256], F32R)
            nc.sync.dma_start(out=ms_sb, in_=ms_d.ap())
            bias_sb = const_pool.tile([128, SUB * 128], BF16)
            nc.sync.dma_start(out=bias_sb, in_=bias_d.ap())

            # Software-pipelined over sub-chunks: transposes run one stage
            # ahead of the matmuls and two ahead of the elementwise tail so
            # the PE's in-order queue never waits on the DVE copy.
            for i in range(N_SUBS + 4):
                # stage A: transposes + PSUM->SBUF copy for sub-chunk i
                if i < N_SUBS:
                    c, sc = divmod(i, SUBS_PER_CHUNK)
                    if sc == 0 and c + 1 < N_CHUNKS:
                        load_chunk(c + 1)
                    x_sb = x_tiles[c]
                    xt_ps = xtp_pool.tile([128, SUB, 128], F32R)     # 1 bank
                    for q in range(SUB):
                        nc.tensor.transpose(xt_ps[:, q, :],
                                            x_sb[:, SUB * sc + q, :], ident)
                    xt_sb = xts_pool.tile([128, SUB, 128], F32R)
                    # Alternate the PSUM->SBUF copy between DVE and ACT to
                    # keep both below the DMA pace.
                    if i % 2 == 0:
                        nc.vector.tensor_copy(xt_sb, xt_ps)
                    else:
                        nc.scalar.copy(xt_sb, xt_ps)
                    xt_tiles[i] = xt_sb

                # stage B: matmuls + square for sub-chunk i-2
                t = i - 2
                if 0 <= t < N_SUBS:
                    xt_sb = xt_tiles.pop(t)
                    mu_ps = mup_pool.tile([128, SUB, 256], F32)      # 2 banks
                    for q in range(SUB):
                        nc.tensor.matmul(mu_ps[:, q, :],
                                         xt_sb[:, q, :], ms_sb,
                                         start=True, stop=True)
                    sq_sb = sq_pool.tile([128, SUB, 256], BF16)
                    # De-interleaving AP pair: reads walk mu x/p interleaved
                    # (stride 2), writes land [x-half | p-half] so the
                    # pair-add reads contiguous halves.
                    mu_v = mu_ps.rearrange("p a b -> p (a b)").rearrange(
                        "p (q m e) -> p q e m", q=SUB, e=2)
                    sq_v = sq_sb.rearrange("p a b -> p (a b)").rearrange(
                        "p (e q m) -> p q e m", e=2, q=SUB)
                    nc.scalar.activation(sq_v, mu_v,
                                         mybir.ActivationFunctionType.Square)
                    mu_tiles[t] = mu_ps
                    sq_tiles[t] = sq_sb

                # stage C: pair-add + bias + output DMA for sub-chunk i-4
                u = i - 4
                if u >= 0:
                    cu, scu = divmod(u, SUBS_PER_CHUNK)
                    mu_tiles.pop(u, None)
                    sq_sb = sq_tiles.pop(u)
                    sq_flat = sq_sb.rearrange("p a b -> p (a b)")
                    tmp_sb = tmp_pool.tile([128, SUB, 128], BF16)
                    tmp_flat = tmp_sb.rearrange("p a b -> p (a b)")
                    nc.vector.tensor_tensor(out=tmp_flat,
                                            in0=sq_flat[:, 0:SUB * 128],
                                            in1=sq_flat[:, SUB * 128:],
                                            op=mybir.AluOpType.add)
                    bias_eng = nc.gpsimd if u % 2 == 0 else nc.vector
                    bias_eng.tensor_tensor(
                        out=out_tiles[cu][:, SUB * scu:SUB * (scu + 1), :],
                        in0=tmp_sb, in1=bias_sb,
                        op=mybir.AluOpType.add)
                    if scu == SUBS_PER_CHUNK - 1:
                        nc.sync.dma_start(out=out_v[cu], in_=out_tiles.pop(cu))
                        x_tiles.pop(cu, None)

    nc.compile()
    return nc


_NC_CACHE = None


def kernel(**inputs: np.ndarray) -> np.ndarray:
    global _NC_CACHE
    X = np.ascontiguousarray(np.asarray(inputs["inputs"], dtype=np.float32))
    params = np.asarray(inputs["params"], dtype=np.float32)
    assert X.shape == (BATCH, N_QUMODES)

    Ms, bias_rep, ident = host_prep(params)

    if _NC_CACHE is None:
        _NC_CACHE = build_bass()
    nc = _NC_CACHE

    in_maps = [
        {"x": X[i * ROWS:(i + 1) * ROWS], "ms": Ms, "bias_rep": bias_rep,
         "ident": ident}
        for i in range(N_CORES)
    ]
    res = run_bass_kernel_spmd(nc, in_maps, core_ids=list(range(N_CORES)))
    out = np.concatenate([r["out"] for r in res.results], axis=0)
    return out.astype(np.float32)
